# revision 17
# baseline (speedup 1.0000x reference)
"""Trainium2 Bass kernel for nn_Executor_48515950576547 (scatter_memory).

Computation (per token t, with K=16 selected pool rows of width D=512):
    sel[t,k,:] = pool_table[indices[t,k], :]
    p[t,k]     = dot(x[t,:], sel[t,k,:])
    tw[t,k]    = tanh(p[t,k]) * weights[t,k]
    out[t,:]   = sum_k tw[t,k] * sel[t,k,:] + x[t,:]

Sharding: data-parallel over the batch dim (B=8 -> one batch row per
NeuronCore), pool_table replicated to every core's HBM. No collectives.

The pool table is converted to bf16 on the host before upload: the
recombine term is ~3% of the residual magnitude, so bf16 rounding of the
gathered rows is far below any fp32 tolerance, while halving both the
HBM gather traffic (1KB/row) and enabling 1-cycle/row PE matmuls.

Per-core kernel layout (2048 tokens, 16 groups of 128):
  - per (group, k): indirect DMA gathers 128 bf16 rows (one per SBUF
    partition) using an int32 index column.
  - products: native InstTensorTensorReduce on VectorE (fused mul+reduce
    along the free dim) -> p[:, k] per-partition scalars.
  - tanh on ScalarE; tw = tanh(p) * w on VectorE.
  - recombine: 16 accumulating bf16 PE matmuls with lhsT = diag(tw[:,k])
    (built on ScalarE as identity * per-partition scalar).
  - residual add (psum f32 + x f32) on VectorE, then DMA out.
"""

import ml_dtypes
import numpy as np

from concourse import bass, mybir
from concourse.bass import IndirectOffsetOnAxis
from concourse.dve_ops import TENSOR_TENSOR_REDUCE
from concourse.bass_utils import run_bass_kernel_spmd
from concourse.masks import make_identity
from concourse.tile import TileContext

B, S, K, D = 8, 2048, 16, 512
POOL = 500000
P = 128
NTOK = S          # tokens per core (one batch row per core)
G = NTOK // P     # 16 groups of 128 tokens
N_CORES = 8

F32 = mybir.dt.float32
BF16 = mybir.dt.bfloat16
I32 = mybir.dt.int32


def _build_kernel(reps: int = 1, mode: str = "full") -> bass.Bass:
    """reps>1 replicates the whole body (identical work) for wall-clock
    benchmarking: per-rep time = (T(reps=R) - T(reps=1)) / (R-1).
    mode: "full" | "gather" (DMA only) | "gather_ttr" (no recombine)."""
    nc = bass.Bass()

    x_d = nc.declare_dram_parameter("x", [NTOK, D], F32, isOutput=False)
    xb_d = nc.declare_dram_parameter("xb", [NTOK, D], BF16, isOutput=False)
    idx_d = nc.declare_dram_parameter("idx", [P, G * K], I32, isOutput=False)
    w_d = nc.declare_dram_parameter("w", [P, G * K], F32, isOutput=False)
    pool_d = nc.declare_dram_parameter("pool", [POOL, D], BF16, isOutput=False)
    out_d = nc.declare_dram_parameter("out", [NTOK, D], F32, isOutput=True)

    with TileContext(nc) as tc:
        with (
            tc.tile_pool(name="const", bufs=1) as constp,
            tc.tile_pool(name="xp", bufs=3) as xp,
            tc.tile_pool(name="selp", bufs=2 * K) as selp,
            tc.tile_pool(name="scp", bufs=2) as scp,
            tc.tile_pool(name="prodp", bufs=2) as prodp,
            tc.tile_pool(name="twp", bufs=2) as twp,
            tc.tile_pool(name="dgp", bufs=4) as dgp,
            tc.tile_pool(name="outp", bufs=3) as outp,
            tc.tile_pool(name="psp", bufs=2, space="PSUM") as psp,
        ):
            identity = constp.tile([P, P], dtype=F32)
            make_identity(nc, identity[:])

            idx_sb = constp.tile([P, G * K], dtype=I32)
            nc.sync.dma_start(out=idx_sb[:], in_=idx_d[:])
            w_sb = constp.tile([P, G * K], dtype=F32)
            nc.sync.dma_start(out=w_sb[:], in_=w_d[:])

            for g in [gg for _ in range(reps) for gg in range(G)]:
                x_t = xp.tile([P, D], dtype=F32, tag="x_t")
                nc.sync.dma_start(out=x_t[:], in_=x_d[g * P : (g + 1) * P, :])
                xb_t = xp.tile([P, D], dtype=BF16, tag="xb_t")
                nc.sync.dma_start(out=xb_t[:], in_=xb_d[g * P : (g + 1) * P, :])

                prod = prodp.tile([P, K], dtype=F32, tag="prod")
                tw = twp.tile([P, K], dtype=F32, tag="tw")
                tw2 = twp.tile([P, K], dtype=F32, tag="tw2")
                ps = psp.tile([P, D], dtype=F32, space="PSUM", tag="ps")
                for k in range(K):
                    sel = selp.tile([P, D], dtype=BF16, tag="sel")
                    c = g * K + k
                    nc.gpsimd.indirect_dma_start(
                        out=sel[:],
                        out_offset=None,
                        in_=pool_d[:],
                        in_offset=IndirectOffsetOnAxis(
                            ap=idx_sb[:, c : c + 1], axis=0
                        ),
                    )
                    if mode == "gather":
                        continue
                    sc = scp.tile([P, D], dtype=BF16, tag="sc")
                    # accum_out = s0 + sum(in0*in1*s1); fp32 accumulation.
                    nc.vector._custom_dve(
                        TENSOR_TENSOR_REDUCE,
                        out=sc[:],
                        in0=sel[:],
                        in1=xb_t[:],
                        s0=0.0,
                        s1=1.0,
                        accum_out=prod[:, k : k + 1],
                    )
                    if mode != "full":
                        continue
                    # Per-k tail: tanh -> *w -> diag -> accumulate matmul.
                    # Keeps the PE fed right behind each TTR and releases
                    # sel slots early for the gather pipeline.
                    nc.scalar.activation(
                        out=tw[:, k : k + 1],
                        in_=prod[:, k : k + 1],
                        func=mybir.ActivationFunctionType.Tanh,
                    )
                    nc.vector.tensor_tensor(
                        out=tw2[:, k : k + 1],
                        in0=tw[:, k : k + 1],
                        in1=w_sb[:, c : c + 1],
                        op=mybir.AluOpType.mult,
                    )
                    dg = dgp.tile([P, P], dtype=BF16, tag="dg")
                    nc.scalar.activation(
                        out=dg[:],
                        in_=identity[:],
                        func=mybir.ActivationFunctionType.Copy,
                        scale=tw2[:, k : k + 1],
                    )
                    nc.tensor.matmul(
                        out=ps[:],
                        lhsT=dg[:],
                        rhs=sel[:],
                        start=(k == 0),
                        stop=(k == K - 1),
                    )

                if mode != "full":
                    out_t = outp.tile([P, D], dtype=F32, tag="out_t")
                    nc.vector.tensor_copy(out=out_t[:], in_=x_t[:])
                    nc.sync.dma_start(
                        out=out_d[g * P : (g + 1) * P, :], in_=out_t[:]
                    )
                    continue

                out_t = outp.tile([P, D], dtype=F32, tag="out_t")
                nc.vector.tensor_tensor(
                    out=out_t[:], in0=ps[:], in1=x_t[:], op=mybir.AluOpType.add
                )
                nc.sync.dma_start(
                    out=out_d[g * P : (g + 1) * P, :], in_=out_t[:]
                )

    # Raw Bass skips Bacc.compile(); run the three passes walrus needs:
    # split multi-waits (HW allows 1 wait/inst), move matmul waits onto
    # ldweights, and populate .instr bytes for extended InstISA subclasses
    # (InstTensorTensorReduce) or walrus sees "ISA wrong length".
    import bass_rust as _bass_rust
    from concourse.library_overlay import lower_extended_insts

    _bass_rust.move_matmul_waits_to_ldweights(nc.m)
    _bass_rust.generate_event_semaphores(nc)
    lower_extended_insts(nc)

    return nc


_NC_CACHE: bass.Bass | None = None
_last_in_maps = None


def _get_nc() -> bass.Bass:
    global _NC_CACHE
    if _NC_CACHE is None:
        _NC_CACHE = _build_kernel()
    return _NC_CACHE


def _make_in_maps(x, indices, weights, pool_table):
    x = np.ascontiguousarray(np.asarray(x, dtype=np.float32))
    indices = np.asarray(indices)
    weights = np.ascontiguousarray(np.asarray(weights, dtype=np.float32))
    pool = np.asarray(pool_table, dtype=np.float32)
    assert x.shape == (B, S, D) and indices.shape == (B, S, K)
    assert weights.shape == (B, S, K) and pool.shape == (POOL, D)

    idx32 = indices.astype(np.int32)
    pool_bf = np.ascontiguousarray(pool.astype(ml_dtypes.bfloat16))
    x_bf = x.astype(ml_dtypes.bfloat16)

    in_maps = []
    for b in range(N_CORES):
        # [P, G*K] layouts: col (g*K + k), partition p  <->  token g*P + p
        idx_t = np.ascontiguousarray(
            idx32[b].reshape(G, P, K).transpose(1, 0, 2).reshape(P, G * K)
        )
        w_t = np.ascontiguousarray(
            weights[b].reshape(G, P, K).transpose(1, 0, 2).reshape(P, G * K)
        )
        in_maps.append(
            {"x": x[b], "xb": x_bf[b], "idx": idx_t, "w": w_t, "pool": pool_bf}
        )
    return in_maps


def kernel(x, indices, weights, pool_table):
    nc = _get_nc()
    in_maps = _make_in_maps(x, indices, weights, pool_table)

    global _last_in_maps
    _last_in_maps = in_maps

    res = run_bass_kernel_spmd(nc, in_maps, core_ids=list(range(N_CORES)))
    out = np.stack([res.results[b]["out"] for b in range(N_CORES)], axis=0)
    return out.astype(np.float32)


# revision 18
# speedup vs baseline: 1.8808x; 1.8808x over previous
"""Trainium2 Bass kernel for nn_Executor_48515950576547 (scatter_memory).

Computation (per token t, with K=16 selected pool rows of width D=512):
    sel[t,k,:] = pool_table[indices[t,k], :]
    p[t,k]     = dot(x[t,:], sel[t,k,:])
    tw[t,k]    = tanh(p[t,k]) * weights[t,k]
    out[t,:]   = sum_k tw[t,k] * sel[t,k,:] + x[t,:]

Sharding: data-parallel over the batch dim (B=8 -> one batch row per
NeuronCore), pool_table replicated to every core's HBM. No collectives.

The pool table is converted to bf16 on the host before upload: the
recombine term is ~3% of the residual magnitude, so bf16 rounding of the
gathered rows is far below any fp32 tolerance, while halving both the
HBM gather traffic (1KB/row) and enabling 1-cycle/row PE matmuls.

Per-core kernel layout (2048 tokens, 16 groups of 128):
  - per (group, k): indirect DMA gathers 128 bf16 rows (one per SBUF
    partition) using an int32 index column.
  - products: native InstTensorTensorReduce on VectorE (fused mul+reduce
    along the free dim) -> p[:, k] per-partition scalars.
  - tanh on ScalarE; tw = tanh(p) * w on VectorE.
  - recombine: 16 accumulating bf16 PE matmuls with lhsT = diag(tw[:,k])
    (built on ScalarE as identity * per-partition scalar).
  - residual add (psum f32 + x f32) on VectorE, then DMA out.
"""

import ml_dtypes
import numpy as np

from concourse import bass, mybir
from concourse.bass import IndirectOffsetOnAxis
from concourse.dve_ops import TENSOR_TENSOR_REDUCE
from concourse.bass_utils import run_bass_kernel_spmd
from concourse.masks import make_identity
from concourse.tile import TileContext

B, S, K, D = 8, 2048, 16, 512
POOL = 500000
P = 128
NTOK = S          # tokens per core (one batch row per core)
G = NTOK // P     # 16 groups of 128 tokens
N_CORES = 8

F32 = mybir.dt.float32
BF16 = mybir.dt.bfloat16
I32 = mybir.dt.int32


def _build_kernel(reps: int = 1, mode: str = "full") -> bass.Bass:
    """reps>1 replicates the whole body (identical work) for wall-clock
    benchmarking: per-rep time = (T(reps=R) - T(reps=1)) / (R-1).
    mode: "full" | "gather" (DMA only) | "gather_ttr" (no recombine)."""
    nc = bass.Bass()

    x_d = nc.declare_dram_parameter("x", [NTOK, D], F32, isOutput=False)
    xb_d = nc.declare_dram_parameter("xb", [NTOK, D], BF16, isOutput=False)
    idx_d = nc.declare_dram_parameter("idx", [P, G * K], I32, isOutput=False)
    w_d = nc.declare_dram_parameter("w", [P, G * K], F32, isOutput=False)
    pool_d = nc.declare_dram_parameter("pool", [POOL, D], BF16, isOutput=False)
    out_d = nc.declare_dram_parameter("out", [NTOK, D], F32, isOutput=True)

    with TileContext(nc) as tc:
        with (
            tc.tile_pool(name="const", bufs=1) as constp,
            tc.tile_pool(name="xp", bufs=3) as xp,
            tc.tile_pool(name="selp", bufs=2 * K) as selp,
            tc.tile_pool(name="scp", bufs=2) as scp,
            tc.tile_pool(name="prodp", bufs=2) as prodp,
            tc.tile_pool(name="twp", bufs=2) as twp,
            tc.tile_pool(name="dgp", bufs=4) as dgp,
            tc.tile_pool(name="outp", bufs=3) as outp,
            tc.tile_pool(name="psp", bufs=2, space="PSUM") as psp,
        ):
            identity = constp.tile([P, P], dtype=F32)
            make_identity(nc, identity[:])

            idx_sb = constp.tile([P, G * K], dtype=I32)
            nc.sync.dma_start(out=idx_sb[:], in_=idx_d[:])
            w_sb = constp.tile([P, G * K], dtype=F32)
            nc.sync.dma_start(out=w_sb[:], in_=w_d[:])

            for g in [gg for _ in range(reps) for gg in range(G)]:
                x_t = xp.tile([P, D], dtype=F32, tag="x_t")
                nc.sync.dma_start(out=x_t[:], in_=x_d[g * P : (g + 1) * P, :])
                xb_t = xp.tile([P, D], dtype=BF16, tag="xb_t")
                nc.sync.dma_start(out=xb_t[:], in_=xb_d[g * P : (g + 1) * P, :])

                prod = prodp.tile([P, K], dtype=F32, tag="prod")
                sels = []
                for k in range(K):
                    sel = selp.tile([P, D], dtype=BF16, tag="sel")
                    c = g * K + k
                    nc.gpsimd.indirect_dma_start(
                        out=sel[:],
                        out_offset=None,
                        in_=pool_d[:],
                        in_offset=IndirectOffsetOnAxis(
                            ap=idx_sb[:, c : c + 1], axis=0
                        ),
                    )
                    sels.append(sel)
                    if mode == "gather":
                        continue
                    sc = scp.tile([P, D], dtype=BF16, tag="sc")
                    # accum_out = s0 + sum(in0*in1*s1); fp32 accumulation.
                    nc.vector._custom_dve(
                        TENSOR_TENSOR_REDUCE,
                        out=sc[:],
                        in0=sel[:],
                        in1=xb_t[:],
                        s0=0.0,
                        s1=1.0,
                        accum_out=prod[:, k : k + 1],
                    )

                if mode != "full":
                    out_t = outp.tile([P, D], dtype=F32, tag="out_t")
                    nc.vector.tensor_copy(out=out_t[:], in_=x_t[:])
                    nc.sync.dma_start(
                        out=out_d[g * P : (g + 1) * P, :], in_=out_t[:]
                    )
                    continue

                # tanh(p) * w
                tw = twp.tile([P, K], dtype=F32, tag="tw")
                nc.scalar.activation(
                    out=tw[:], in_=prod[:], func=mybir.ActivationFunctionType.Tanh
                )
                tw2 = twp.tile([P, K], dtype=F32, tag="tw2")
                nc.vector.tensor_tensor(
                    out=tw2[:],
                    in0=tw[:],
                    in1=w_sb[:, g * K : (g + 1) * K],
                    op=mybir.AluOpType.mult,
                )

                # out2 = sum_k diag(tw[:,k]) @ sel_k, accumulated in PSUM
                ps = psp.tile([P, D], dtype=F32, space="PSUM", tag="ps")
                for k in range(K):
                    dg = dgp.tile([P, P], dtype=BF16, tag="dg")
                    nc.scalar.activation(
                        out=dg[:],
                        in_=identity[:],
                        func=mybir.ActivationFunctionType.Copy,
                        scale=tw2[:, k : k + 1],
                    )
                    nc.tensor.matmul(
                        out=ps[:],
                        lhsT=dg[:],
                        rhs=sels[k][:],
                        start=(k == 0),
                        stop=(k == K - 1),
                    )

                out_t = outp.tile([P, D], dtype=F32, tag="out_t")
                nc.vector.tensor_tensor(
                    out=out_t[:], in0=ps[:], in1=x_t[:], op=mybir.AluOpType.add
                )
                nc.sync.dma_start(
                    out=out_d[g * P : (g + 1) * P, :], in_=out_t[:]
                )

    # Raw Bass skips Bacc.compile(); run the three passes walrus needs:
    # split multi-waits (HW allows 1 wait/inst), move matmul waits onto
    # ldweights, and populate .instr bytes for extended InstISA subclasses
    # (InstTensorTensorReduce) or walrus sees "ISA wrong length".
    import bass_rust as _bass_rust
    from concourse.library_overlay import lower_extended_insts

    _bass_rust.move_matmul_waits_to_ldweights(nc.m)
    _bass_rust.generate_event_semaphores(nc)
    lower_extended_insts(nc)

    return nc


_NC_CACHE: bass.Bass | None = None
_last_in_maps = None


def _get_nc() -> bass.Bass:
    global _NC_CACHE
    if _NC_CACHE is None:
        _NC_CACHE = _build_kernel()
    return _NC_CACHE


def _make_in_maps(x, indices, weights, pool_table):
    x = np.ascontiguousarray(np.asarray(x, dtype=np.float32))
    indices = np.asarray(indices)
    weights = np.ascontiguousarray(np.asarray(weights, dtype=np.float32))
    pool = np.asarray(pool_table, dtype=np.float32)
    assert x.shape == (B, S, D) and indices.shape == (B, S, K)
    assert weights.shape == (B, S, K) and pool.shape == (POOL, D)

    idx32 = indices.astype(np.int32)
    pool_bf = np.ascontiguousarray(pool.astype(ml_dtypes.bfloat16))
    x_bf = x.astype(ml_dtypes.bfloat16)

    in_maps = []
    for b in range(N_CORES):
        # [P, G*K] layouts: col (g*K + k), partition p  <->  token g*P + p
        idx_t = np.ascontiguousarray(
            idx32[b].reshape(G, P, K).transpose(1, 0, 2).reshape(P, G * K)
        )
        w_t = np.ascontiguousarray(
            weights[b].reshape(G, P, K).transpose(1, 0, 2).reshape(P, G * K)
        )
        in_maps.append(
            {"x": x[b], "xb": x_bf[b], "idx": idx_t, "w": w_t, "pool": pool_bf}
        )
    return in_maps


def kernel(x, indices, weights, pool_table):
    nc = _get_nc()
    in_maps = _make_in_maps(x, indices, weights, pool_table)

    global _last_in_maps
    _last_in_maps = in_maps

    res = run_bass_kernel_spmd(nc, in_maps, core_ids=list(range(N_CORES)))
    out = np.stack([res.results[b]["out"] for b in range(N_CORES)], axis=0)
    return out.astype(np.float32)
